# revision 22
# baseline (speedup 1.0000x reference)
"""Trainium2 Bass kernel for nn_DNN_Model_33852932227151.

Per-sample pipeline (see reference):
  theta1 = MLP(sample1)            303 -> 1024 -> 1024 -> 512 -> 264
  F1, F2 normalized precoders      (cols 200:264)
  theta  = unit-modulus phases     (cols 0:200 as complex [100])
  CCC_bc = Re(theta^H T_bc theta) / 1e-15 ; scale = rsqrt(max(max_c CCC, 1))
  out    = [Re(theta*scale), Im(theta*scale), Re F1, Im F1, Re F2, Im F2]

Sharding: pure data parallel over batch: 2048 = 8 cores x 256 samples.
Weights replicated. Inside a core:
  - MLP on TensorE, feature-major activations [feat, 256].
  - PE transposes to sample-major [128, *] for per-sample math.
  - Re(theta^H T theta) = <Tr, aa^T+bb^T>_F + <Ti, ba^T-ab^T>_F computed by
    building outer-product tiles (DVE) with samples on partitions and fusing
    multiply+reduce over streamed T chunks via tensor_tensor_reduce.
The kernel is HBM-bandwidth bound on streaming T (82 MB/core).
"""

import os
import threading

import numpy as np

import concourse.bass as bass
from concourse import bacc
import concourse.mybir as mybir
import concourse.tile as tile
from concourse.bass_utils import run_bass_kernel_spmd

F32 = mybir.dt.float32
BF16 = mybir.dt.bfloat16

# ---- problem constants (hardcoded per harness contract) ----
B = 2048
N_CORES = 8
B_LOC = B // N_CORES          # 256 samples per core
DIN = 303
H1, H2, H3 = 1024, 1024, 512
DOUT = 264
NRIS = 100
C = 4
MN = 16
INV_THRESH = 1.0e15           # 1 / THRESH_W

N_GROUPS = B_LOC // 128       # 2 sample groups of 128 per core
ROWS = 25                     # RIS rows per T chunk
N_H = NRIS // ROWS            # 4 chunks per (c, group)
CHUNK = ROWS * NRIS           # 2500 elements per partition per chunk

# contraction dtype for the big T inner product ("f32" safest, "bf16" fastest)
CONTRACT_DT = os.environ.get("KERNEL_CONTRACT_DT", "f32")
# T chunk load: "hwdge" = plain fp32 DMA (+ACT cast if bf16), "cast" = SWDGE cast DMA
T_LOAD = os.environ.get("KERNEL_T_LOAD", "hwdge")
# debug bisect: "full" | "noquad" (skip T contraction; scale=1)
#               | "nottr" (stream T + build outers, but fake CCC)
STAGE = os.environ.get("KERNEL_STAGE", "full")


def _ceil_div(a, b):
    return (a + b - 1) // b


def build_nc():
    nc = bacc.Bacc(trn_type="TRN2", debug=False)

    # ---- DRAM parameters (per-core shard shapes) ----
    s1 = nc.declare_dram_parameter("sample1", [B_LOC, DIN], F32, isOutput=False)
    t_re = nc.declare_dram_parameter("T_real", [B_LOC, C, NRIS, NRIS], F32, isOutput=False)
    t_im = nc.declare_dram_parameter("T_imag", [B_LOC, C, NRIS, NRIS], F32, isOutput=False)
    w1 = nc.declare_dram_parameter("W1", [DIN, H1], F32, isOutput=False)
    b1 = nc.declare_dram_parameter("b1", [H1], F32, isOutput=False)
    w2 = nc.declare_dram_parameter("W2", [H1, H2], F32, isOutput=False)
    b2 = nc.declare_dram_parameter("b2", [H2], F32, isOutput=False)
    w3 = nc.declare_dram_parameter("W3", [H2, H3], F32, isOutput=False)
    b3 = nc.declare_dram_parameter("b3", [H3], F32, isOutput=False)
    w4 = nc.declare_dram_parameter("W4", [H3, DOUT], F32, isOutput=False)
    b4 = nc.declare_dram_parameter("b4", [DOUT], F32, isOutput=False)
    out = nc.declare_dram_parameter("out", [B_LOC, DOUT], F32, isOutput=True)

    ident_dram = nc.inline_tensor(np.eye(128, dtype=np.float32), name="ident128")

    cdt = BF16 if CONTRACT_DT == "bf16" else F32

    with tile.TileContext(nc) as tc:
        _emit(tc, s1, t_re, t_im, (w1, b1), (w2, b2), (w3, b3), (w4, b4),
              out, ident_dram, cdt)
    nc.compile()
    return nc


def _emit(tc, s1, t_re, t_im, l1, l2, l3, l4, out, ident_dram, cdt):
    nc = tc.nc
    w1, b1 = l1
    w2, b2 = l2
    w3, b3 = l3
    w4, b4 = l4

    with (
        tc.tile_pool(name="consts", bufs=1) as consts,
        tc.tile_pool(name="acts", bufs=1) as acts,
        tc.tile_pool(name="theta", bufs=1) as theta_pool,
        tc.tile_pool(name="tch", bufs=4) as tch_pool,
        tc.tile_pool(name="tsc", bufs=2) as tsc_pool,
        tc.tile_pool(name="psmm", bufs=4, space="PSUM") as psmm,
        tc.tile_pool(name="pstr", bufs=2, space="PSUM") as pstr,
    ):
        ident = consts.tile([128, 128], F32)
        nc.sync.dma_start(out=ident, in_=ident_dram[:, :])

        # ---------------- MLP (feature-major) ----------------
        # Weight/bias staging. Weights + MLP activations live in their own
        # pool so SBUF frees before the contraction phase peaks.
        with tc.tile_pool(name="weights", bufs=1) as wpool:
            w1s = wpool.tile([128, 3, H1], F32)
            nc.vector.memset(w1s[:, 2, :], 0.0)
            nc.sync.dma_start(out=w1s[:, 0, :], in_=w1[0:128, :])
            nc.sync.dma_start(out=w1s[:, 1, :], in_=w1[128:256, :])
            nc.sync.dma_start(out=w1s[0:47, 2, :], in_=w1[256:303, :])
            w2s = wpool.tile([128, 8, H2], F32)
            nc.sync.dma_start(out=w2s, in_=w2[:, :].rearrange("(o p) m -> p o m", p=128))
            w3s = wpool.tile([128, 8, H3], F32)
            nc.sync.dma_start(out=w3s, in_=w3[:, :].rearrange("(o p) m -> p o m", p=128))
            w4s = wpool.tile([128, 4, DOUT], F32)
            nc.sync.dma_start(out=w4s, in_=w4[:, :].rearrange("(o p) m -> p o m", p=128))
            # biases, feature-on-partition layout per m-chunk
            b1s = wpool.tile([128, 8], F32)
            nc.sync.dma_start(out=b1s, in_=b1[:].rearrange("(o p) -> p o", p=128))
            b2s = wpool.tile([128, 8], F32)
            nc.sync.dma_start(out=b2s, in_=b2[:].rearrange("(o p) -> p o", p=128))
            b3s = wpool.tile([128, 4], F32)
            nc.sync.dma_start(out=b3s, in_=b3[:].rearrange("(o p) -> p o", p=128))
            b4s = wpool.tile([128, 3], F32)
            nc.sync.dma_start(out=b4s[0:100, 0:1], in_=b4[0:100, None])
            nc.sync.dma_start(out=b4s[0:100, 1:2], in_=b4[100:200, None])
            nc.sync.dma_start(out=b4s[0:64, 2:3], in_=b4[200:264, None])

            # Input transpose: sample1 [256, 303] -> X0 [128, 3, 256] feature-major
            x0 = wpool.tile([128, 3, B_LOC], F32)
            nc.vector.memset(x0[:, 2, :], 0.0)
            for bt in range(2):
                s1_nat = tsc_pool.tile([128, DIN], F32, tag="s1nat")
                nc.sync.dma_start(out=s1_nat, in_=s1[bt * 128:(bt + 1) * 128, :])
                for ft in range(3):
                    w = min(128, DIN - ft * 128)
                    ps = pstr.tile([128, 128], F32, tag="tr")
                    nc.tensor.transpose(ps[0:w, :], s1_nat[:, ft * 128:ft * 128 + w],
                                        ident)
                    nc.scalar.copy(out=x0[0:w, ft, bt * 128:(bt + 1) * 128],
                                   in_=ps[0:w, :])

            def dense(in_tile, n_k, ws, n_m, bias_s, relu, out_tile, m_widths=None):
                # out[feat, batch] = act(W.T @ in + b); K = n_k*128 on partitions
                for mo in range(n_m):
                    if m_widths is None:
                        mw, m_lo = 128, mo * 128
                    else:
                        mw, m_lo = m_widths[mo][1], m_widths[mo][0]
                    ps = psmm.tile([128, B_LOC], F32, tag="mm")
                    for k in range(n_k):
                        nc.tensor.matmul(
                            ps[0:mw, :],
                            lhsT=ws[:, k, m_lo:m_lo + mw],
                            rhs=in_tile[:, k, :],
                            start=(k == 0),
                            stop=(k == n_k - 1),
                        )
                    if relu:
                        # out = relu(psum + bias), bias per-partition
                        nc.scalar.activation(
                            out=out_tile[0:mw, mo, :], in_=ps[0:mw, :],
                            func=mybir.ActivationFunctionType.Relu,
                            bias=bias_s[0:mw, mo:mo + 1], scale=1.0)
                    else:
                        nc.vector.tensor_scalar(
                            out=out_tile[0:mw, mo, :], in0=ps[0:mw, :],
                            scalar1=bias_s[0:mw, mo:mo + 1], scalar2=None,
                            op0=mybir.AluOpType.add)

            h1t = wpool.tile([128, 8, B_LOC], F32)
            dense(x0, 3, w1s, 8, b1s, True, h1t)
            h2t = wpool.tile([128, 8, B_LOC], F32)
            dense(h1t, 8, w2s, 8, b2s, True, h2t)
            h3t = wpool.tile([128, 4, B_LOC], F32)
            dense(h2t, 8, w3s, 4, b3s, True, h3t)
            # Layer 4 (no relu), output split in aligned chunks:
            #   [0:100] Re(theta_raw), [100:200] Im(theta_raw), [200:264] F
            thp = acts.tile([128, 3, B_LOC], F32)  # [:,0]=re, [:,1]=im, [:,2]=F(64)
            dense(h3t, 4, w4s, 3, b4s, False, thp,
                  m_widths=[(0, 100), (100, 100), (200, 64)])

        # ---------------- unit-modulus theta (feature-major) ----------------
        p_re = thp[0:100, 0, :]
        p_im = thp[0:100, 1, :]
        sq = tsc_pool.tile([100, B_LOC], F32, tag="sq")
        sq2 = tsc_pool.tile([100, B_LOC], F32, tag="sq2")
        nc.vector.tensor_mul(sq, p_re, p_re)
        nc.vector.tensor_mul(sq2, p_im, p_im)
        nc.vector.tensor_add(sq, sq, sq2)
        nc.scalar.sqrt(sq, sq)
        nc.vector.reciprocal(sq, sq)               # sq = 1/|theta|
        # [128, *] tiles so PE transposes use a full K=128 contraction
        # (sub-128-K matmuls misbehave); rows 100:128 are garbage that the
        # post-transpose column slice discards.
        a_fm = theta_pool.tile([128, B_LOC], F32)  # Re(theta), unit modulus
        b_fm = theta_pool.tile([128, B_LOC], F32)  # Im(theta)
        nc.vector.tensor_mul(a_fm[0:100, :], p_re, sq)
        nc.vector.tensor_mul(b_fm[0:100, :], p_im, sq)

        # ---------------- per-group sample-major processing ----------------
        # obuild opens after the weights pool closed: its arena reuses the
        # freed weight space so peak SBUF stays under the Tile cap.
        obuild_cm = tc.tile_pool(name="obuild", bufs=1)
        obuild = obuild_cm.__enter__()
        ccc_all = theta_pool.tile([128, N_GROUPS, C], F32)
        for g in range(N_GROUPS):
            gs = slice(g * 128, (g + 1) * 128)

            def to_sample_major(src_fm, np_, tag):
                # src [128, 128] feature-major slice -> [128, np_] sample-major
                # (full K=128 transpose; columns np_:128 are garbage, dropped)
                ps = pstr.tile([128, 128], F32, tag="tr")
                nc.tensor.transpose(ps, src_fm, ident)
                dst = theta_pool.tile([128, np_], F32, tag=tag)
                nc.scalar.copy(out=dst, in_=ps[:, 0:np_])
                return dst

            a_pack = to_sample_major(a_fm[:, gs], 100, f"apack{g}")
            b_pack = to_sample_major(b_fm[:, gs], 100, f"bpack{g}")
            f_pack = to_sample_major(thp[:, 2, gs], 64, f"fpack{g}")

            # ---- F1/F2 precoder normalization (sample-major) ----
            fsq = tsc_pool.tile([128, 2, 32], F32, tag="fsq")
            f_v = f_pack[:].rearrange("p (g2 i) -> p g2 i", g2=2)
            nc.vector.tensor_mul(fsq, f_v, f_v)
            fnorm = tsc_pool.tile([128, 2], F32, tag="fnorm")
            nc.vector.reduce_sum(fnorm, fsq, axis=mybir.AxisListType.X)
            # scale = sqrt(2/norm) = 1/sqrt(norm*0.5)
            nc.scalar.activation(out=fnorm, in_=fnorm,
                                 func=mybir.ActivationFunctionType.Sqrt, scale=0.5)
            nc.vector.reciprocal(fnorm, fnorm)
            fhat = theta_pool.tile([128, 2, 32], F32, tag=f"fhat{g}")
            nc.vector.tensor_mul(fhat, f_v,
                                 fnorm[:, :, None].to_broadcast((128, 2, 32)))
            nc.sync.dma_start(out=out[gs, 200:264],
                              in_=fhat[:].rearrange("p g2 i -> p (g2 i)"))

            if STAGE == "noquad":
                nc.sync.dma_start(out=out[gs, 0:100], in_=a_pack)
                nc.sync.dma_start(out=out[gs, 100:200], in_=b_pack)
                continue

            # ---- outer products in contraction dtype ----
            if cdt != F32:
                a_c = theta_pool.tile([128, NRIS], cdt, tag=f"ac{g}")
                b_c = theta_pool.tile([128, NRIS], cdt, tag=f"bc{g}")
                nc.scalar.copy(out=a_c, in_=a_pack)
                nc.scalar.copy(out=b_c, in_=b_pack)
            else:
                a_c, b_c = a_pack, b_pack
            s_c = theta_pool.tile([128, NRIS], cdt, tag=f"sc{g}")
            nc.vector.tensor_add(s_c, a_c, b_c)

            o1 = obuild.tile([128, N_H, ROWS, NRIS], cdt, tag="o1")
            o2 = obuild.tile([128, N_H, ROWS, NRIS], cdt, tag="o2")
            for h in range(N_H):
                hs = slice(h * ROWS, (h + 1) * ROWS)
                sh3 = (128, ROWS, NRIS)
                t_ab = obuild.tile([128, ROWS, NRIS], cdt, tag="tab")
                nc.vector.tensor_mul(t_ab, a_c[:, hs, None].to_broadcast(sh3),
                                     b_c[:, None, :].to_broadcast(sh3))
                # o2[h] starts as tT (b_n a_m), o1[h] as ss
                nc.vector.tensor_mul(o2[:, h], b_c[:, hs, None].to_broadcast(sh3),
                                     a_c[:, None, :].to_broadcast(sh3))
                nc.vector.tensor_mul(o1[:, h], s_c[:, hs, None].to_broadcast(sh3),
                                     s_c[:, None, :].to_broadcast(sh3))
                # o1 = ss - t - tT  (= aa^T + bb^T)
                nc.vector.tensor_tensor(o1[:, h], o1[:, h], t_ab,
                                        mybir.AluOpType.subtract)
                nc.vector.tensor_tensor(o1[:, h], o1[:, h], o2[:, h],
                                        mybir.AluOpType.subtract)
                # o2 = tT - t   (Re(quad) = <Tr,o1> + <Ti,o2>)
                nc.vector.tensor_tensor(o2[:, h], o2[:, h], t_ab,
                                        mybir.AluOpType.subtract)

            # ---- stream T; per chunk: DVE multiply, ACT accumulate-reduce ----
            # (tensor_tensor_reduce crashes this HW stack, so the reduce
            # rides scalar-engine activation(Copy, accum_out=...) instead)
            act_dump = obuild.tile([128, ROWS, NRIS], cdt, tag="dump")
            parts = theta_pool.tile([128, C, 2 * N_H], F32, tag=f"parts{g}")
            ccc = ccc_all[:, g, :]
            for c in range(C):
                for ri, (t_dram, o_t) in enumerate(((t_re, o1), (t_im, o2))):
                    for h in range(N_H):
                        chunk = tch_pool.tile([128, ROWS, NRIS], cdt, tag="tchunk")
                        src = t_dram[gs, c, h * ROWS:(h + 1) * ROWS, :]
                        if cdt == F32:
                            nc.sync.dma_start(out=chunk, in_=src)
                        else:
                            # SWDGE cast-DMA: fp32 HBM -> bf16 SBUF in-flight
                            nc.gpsimd.dma_start(out=chunk, in_=src)
                        if STAGE == "nottr":
                            nc.vector.tensor_copy(out=act_dump, in_=chunk)
                            continue
                        prod = tch_pool.tile([128, ROWS, NRIS], cdt, tag="prod")
                        nc.vector.tensor_mul(prod, chunk, o_t[:, h])
                        k = ri * N_H + h
                        nc.scalar.activation(
                            out=act_dump, in_=prod,
                            func=mybir.ActivationFunctionType.Copy,
                            bias=0.0, scale=INV_THRESH,
                            accum_out=parts[:, c, k:k + 1])
            if STAGE == "nottr":
                nc.vector.memset(ccc, 1.0e14)
            else:
                nc.vector.reduce_sum(ccc, parts, axis=mybir.AxisListType.X)

            # ---- scale = rsqrt(max(max_c CCC, 1)) ; theta_hat ----
            mx = tsc_pool.tile([128, 1], F32, tag="mx")
            nc.vector.reduce_max(mx, ccc, axis=mybir.AxisListType.X)
            nc.vector.tensor_scalar(out=mx, in0=mx, scalar1=1.0, scalar2=None,
                                    op0=mybir.AluOpType.max)
            nc.scalar.sqrt(mx, mx)
            nc.vector.reciprocal(mx, mx)
            th_re = theta_pool.tile([128, NRIS], F32, tag=f"thre{g}")
            th_im = theta_pool.tile([128, NRIS], F32, tag=f"thim{g}")
            nc.vector.tensor_scalar_mul(th_re, a_pack, mx)
            nc.vector.tensor_scalar_mul(th_im, b_pack, mx)
            nc.sync.dma_start(out=out[gs, 0:100], in_=th_re)
            nc.sync.dma_start(out=out[gs, 100:200], in_=th_im)
        obuild_cm.__exit__(None, None, None)


_NC_LOCK = threading.Lock()
_NC = None


def _get_nc():
    global _NC
    with _NC_LOCK:
        if _NC is None:
            _NC = build_nc()
    return _NC


def _shard_inputs(inputs):
    in_maps = []
    for i in range(N_CORES):
        bs = slice(i * B_LOC, (i + 1) * B_LOC)
        in_maps.append({
            "sample1": np.ascontiguousarray(inputs["sample1"][bs]),
            "T_real": np.ascontiguousarray(inputs["T_real"][bs]),
            "T_imag": np.ascontiguousarray(inputs["T_imag"][bs]),
            "W1": np.asarray(inputs["W1"]), "b1": np.asarray(inputs["b1"]),
            "W2": np.asarray(inputs["W2"]), "b2": np.asarray(inputs["b2"]),
            "W3": np.asarray(inputs["W3"]), "b3": np.asarray(inputs["b3"]),
            "W4": np.asarray(inputs["W4"]), "b4": np.asarray(inputs["b4"]),
        })
    return in_maps


def run_on_hw(inputs, trace=False, **kwargs):
    nc = _get_nc()
    res = run_bass_kernel_spmd(nc, _shard_inputs(inputs),
                               list(range(N_CORES)), trace=trace, **kwargs)
    full = np.concatenate([res.results[i]["out"] for i in range(N_CORES)], axis=0)
    return full, res


def kernel(**inputs) -> np.ndarray:
    full, _ = run_on_hw(inputs, trace=False)
    return full.astype(np.float32)
